# revision 38
# baseline (speedup 1.0000x reference)
"""Cross-attention kernel for 8 Trainium2 NeuronCores.

Problem: out = softmax((x@Wq)(ctx@Wk)^T * dh^-0.5) @ (ctx@Wv) @ Wo + bo
  x [16, 4096, 320], ctx [16, 77, 768], H=8 heads x DH=40.

Sharding: data-parallel over batch (2 per core), SPMD one NEFF.

Layout ("^T domain": features on partitions, tokens on free dim), bf16
on-chip with fp32 PSUM accumulation:
  x^T, ctx^T   <- transposed + bf16-converted ON HOST, DMA'd directly
  q^T          <- Wq.T @ x^T, dense 3-group tiles {128,128,64}
  S_h          <- k^T_h.T @ q^T_h  [77 ctx-tok, 512 q-tok], pairs share
                  a 2-bank PSUM tile -> one exp per pair (ACT, bf16 out)
  Zs           <- accumulated selector-column matmuls [8, 512]
  Rs           <- exp(-ln(Zs))     (ACT; 1/Z without reciprocal)
  Rb           <- E_pair.T @ Rs    broadcast R rows to pair layout (PSUM)
  O_pair       <- v_h.T @ P_h (64-aligned pair rows), then ONE DVE
                  tensor_tensor per pair reading both PSUM operands:
                  o_sb = O_pair * Rb  (normalized, bf16)
  out          <- o_sb.T @ Wo_pad (+ bo) -> [tokens, 320] fp32 -> DMA

Matmul cost on PE is out-free-rows; bf16 keeps 1 cyc/row everywhere and
halves DMA traffic for x; final copies ride the idle Pool engine.
"""

import numpy as np

H, DH = 8, 40
SCALE = DH ** -0.5
B, N, M = 16, 4096, 77
QD, CD, ID = 320, 768, H * DH
N_CORES = 8
B_LOC = B // N_CORES
CHUNK = 512
NCHUNK = N // CHUNK
NPAIR = H // 2
MP = 80  # ctx tokens padded

_cache = {}


def _legalize_sync_waits(nc, mybir):
    """This walrus build allows 1 sync-wait command per instruction (2 for
    EventSemaphore). Spill extra waits onto same-engine NoOps placed just
    before; per-engine program order makes that equivalent."""
    n = 0
    f = nc.m.functions[0]
    for blk in f.blocks:
        out = []
        changed = False
        for inst in blk.instructions:
            si = inst.sync_info
            waits = list(si.on_wait) if si is not None and si.on_wait else []
            cap = 2 if isinstance(inst, mybir.InstEventSemaphore) else 1
            if len(waits) > cap:
                keep, spill = waits[-cap:], waits[:-cap]
                for w in spill:
                    n += 1
                    nop = mybir.InstNoOp(name=f"I-waitfix-{n}", ins=[], outs=[],
                                         engine=inst.engine)
                    nop.sync_info = mybir.SyncInfo(on_wait=[w], on_update=[])
                    out.append(nop)
                inst.sync_info = mybir.SyncInfo(
                    on_wait=keep,
                    on_update=list(si.on_update) if si.on_update else [])
                changed = True
            out.append(inst)
        if changed:
            blk.instructions = out
    return n


def _bf16(a):
    import ml_dtypes
    return np.asarray(a, dtype=np.float32).astype(ml_dtypes.bfloat16)


def _aux_arrays(x, context, Wq, Wk, Wv, Wo, bo):
    """Host-side prep: transpose x/ctx, convert weights, pair maps."""
    xt = _bf16(np.ascontiguousarray(np.transpose(x, (0, 2, 1))))  # [B,320,4096]
    ctp = np.zeros((B, CD, MP), dtype=np.float32)
    ctp[:, :, :M] = np.transpose(context, (0, 2, 1))

    def pad_pairs_cols(W):
        # W [c, 320] -> [c, 4, 128]: cols 0:40 = head 2p, 64:104 = head 2p+1
        c = W.shape[0]
        out = np.zeros((c, NPAIR, 128), dtype=np.float32)
        for p in range(NPAIR):
            out[:, p, 0:40] = W[:, 80 * p: 80 * p + 40]
            out[:, p, 64:104] = W[:, 80 * p + 40: 80 * p + 80]
        return out.reshape(c, NPAIR * 128)

    # E: [8, 4*128] broadcast map R_h -> pair rows
    e_mat = np.zeros((H, NPAIR * 128), dtype=np.float32)
    for p in range(NPAIR):
        e_mat[2 * p, 128 * p: 128 * p + 40] = 1.0
        e_mat[2 * p + 1, 128 * p + 64: 128 * p + 104] = 1.0
    # selector columns for Zs accumulation: [77, 8], col h = ones
    zcol = np.zeros((M, H, H), dtype=np.float32)
    for h in range(H):
        zcol[:, h, h] = 1.0
    zcol = zcol.reshape(M, H * H)
    # Wo rows in pair layout: [4, 128, 320]
    wo_pad = np.zeros((NPAIR, 128, QD), dtype=np.float32)
    for p in range(NPAIR):
        wo_pad[p, 0:40] = Wo[80 * p: 80 * p + 40]
        wo_pad[p, 64:104] = Wo[80 * p + 40: 80 * p + 80]
    return {
        "xt": xt, "ctp": _bf16(ctp),
        "aux_wq": _bf16(pad_pairs_cols(Wq)), "aux_wk": _bf16(pad_pairs_cols(Wk)),
        "aux_wv": _bf16(Wv),
        "aux_wo": _bf16(wo_pad.reshape(NPAIR * 128, QD)),
        "aux_e": _bf16(e_mat), "aux_z": _bf16(zcol),
        "aux_bo": bo.reshape(1, QD).astype(np.float32),
    }


def _build(nc, with_bias):
    import concourse.mybir as mybir
    from concourse.tile import TileContext

    F = mybir.dt.float32
    BF = mybir.dt.bfloat16
    AF = mybir.ActivationFunctionType

    xt_d = nc.dram_tensor("xt", [B_LOC, QD, N], BF, kind="ExternalInput")
    ct_d = nc.dram_tensor("ctp", [B_LOC, CD, MP], BF, kind="ExternalInput")
    wq_d = nc.dram_tensor("aux_wq", [QD, 512], BF, kind="ExternalInput")
    wk_d = nc.dram_tensor("aux_wk", [CD, 512], BF, kind="ExternalInput")
    wv_d = nc.dram_tensor("aux_wv", [CD, QD], BF, kind="ExternalInput")
    wo_d = nc.dram_tensor("aux_wo", [512, QD], BF, kind="ExternalInput")
    e_d = nc.dram_tensor("aux_e", [H, 512], BF, kind="ExternalInput")
    z_d = nc.dram_tensor("aux_z", [M, H * H], BF, kind="ExternalInput")
    bo_d = nc.dram_tensor("aux_bo", [1, QD], F, kind="ExternalInput")
    out_d = nc.dram_tensor("out", [B_LOC, N, QD], F, kind="ExternalOutput")

    GT = [(0, 128), (128, 128), (256, 64)]          # QD group/k tiles
    CKT6 = [(128 * i, 128) for i in range(6)]       # CD k-tiles

    with TileContext(nc) as tc:
        with tc.tile_pool(name="wpool", bufs=1) as wp, \
             tc.tile_pool(name="bpool", bufs=2) as bp, \
             tc.tile_pool(name="cpool", bufs=2) as cp, \
             tc.tile_pool(name="epool", bufs=10) as ep, \
             tc.tile_pool(name="ps", bufs=2, space="PSUM") as ps:

            # ---- per-core constants ----
            wq_r = []
            for kt, (o, w) in enumerate(GT):
                t = wp.tile([w, 512], BF, name=f"wq{kt}", tag=f"wq{kt}")
                nc.sync.dma_start(t[:], wq_d[o:o + w, :])
                wq_r.append(t)
            ctx_tiles = []
            for b in range(B_LOC):
                t = bp.tile([128, 6, MP], BF, name=f"ctx{b}", tag="ctx")
                nc.sync.dma_start(
                    t[:], ct_d[b].rearrange("(k p) m -> p k m", p=128))
                ctx_tiles.append(t)
            wk_r = []
            wv_r = []
            for kt, (o, w) in enumerate(CKT6):
                t = wp.tile([w, 512], BF, name=f"wk{kt}", tag=f"wk{kt}")
                nc.gpsimd.dma_start(t[:], wk_d[o:o + w, :])
                wk_r.append(t)
                t2 = wp.tile([w, QD], BF, name=f"wv{kt}", tag=f"wv{kt}")
                nc.gpsimd.dma_start(t2[:], wv_d[o:o + w, :])
                wv_r.append(t2)
            wo_r = []
            for p in range(NPAIR):
                t = wp.tile([128, QD], BF, name=f"wo{p}", tag=f"wo{p}")
                nc.gpsimd.dma_start(t[:], wo_d[128 * p:128 * p + 128, :])
                wo_r.append(t)
            e_r = wp.tile([H, 512], BF, name="e_r", tag="e_r")
            nc.gpsimd.dma_start(e_r[:], e_d[:])
            z_r = wp.tile([M, H * H], BF, name="z_r", tag="z_r")
            nc.gpsimd.dma_start(z_r[:], z_d[:])
            if with_bias:
                bo_r = wp.tile([1, QD], F, name="bo_r", tag="bo_r")
                nc.gpsimd.dma_start(bo_r[:], bo_d[:])
                ones_r = wp.tile([1, 128], F, name="ones_r", tag="ones_r")
                nc.vector.memset(ones_r[:], 1.0)

            def batch_prep(b):
                # ---- per-batch: k^T pairs, v ----
                ctx_f = ctx_tiles[b]
                kT_r = []
                for p in range(NPAIR):
                    pt = ps.tile([128, MP], F, name=f"kps{p}", tag="q")
                    for kt in range(6):
                        nc.tensor.matmul(pt[:], wk_r[kt][:, 128 * p:128 * (p + 1)],
                                         ctx_f[:, kt, :], start=(kt == 0),
                                         stop=(kt == 5))
                    st = bp.tile([128, MP], BF, name=f"kT{p}", tag=f"kT{p}")
                    nc.vector.tensor_copy(st[:], pt[:])
                    kT_r.append(st)

                v_ps = ps.tile([MP, ID], F, name="v_ps", tag="q")
                for kt in range(6):
                    nc.tensor.matmul(v_ps[:], ctx_f[:, kt, 0:MP], wv_r[kt][:],
                                     start=(kt == 0), stop=(kt == 5))
                v_r = bp.tile([M, H, 64], BF, name="v_r", tag="v_r")
                nc.vector.memset(v_r[:, :, 40:64], 0.0)
                nc.vector.tensor_copy(
                    v_r[:, :, 0:40],
                    v_ps[0:M, :].rearrange("p (h d) -> p h d", h=H))
                return kT_r, v_r

            prep = {0: batch_prep(0)}
            for b in range(B_LOC):
                kT_r, v_r = prep[b]
                for c in range(NCHUNK):
                    t0 = CHUNK * c
                    if c == NCHUNK - 2 and b + 1 < B_LOC:
                        prep[b + 1] = batch_prep(b + 1)
                    # ---- x^T tiles straight from HBM ----
                    xT_r = []
                    for g, (go, gw) in enumerate(GT):
                        st = cp.tile([gw, CHUNK], BF, name=f"xT{g}",
                                     tag=f"xT{g}")
                        nc.sync.dma_start(st[:], xt_d[b, go:go + gw,
                                                      t0:t0 + CHUNK])
                        xT_r.append(st)

                    # ---- q^T head pairs ----
                    qT_r = []
                    for p in range(NPAIR):
                        pt = ps.tile([128, CHUNK], F, name=f"qps{p}", tag="q")
                        for kt in range(3):
                            nc.tensor.matmul(pt[:], wq_r[kt][:, 128 * p:128 * (p + 1)],
                                             xT_r[kt][:], start=(kt == 0),
                                             stop=(kt == 2))
                        st = cp.tile([128, CHUNK], BF, name=f"qT{p}",
                                     tag=f"qT{p}")
                        nc.vector.tensor_copy(st[:], pt[:])
                        qT_r.append(st)

                    # ---- scores + exp per head ----
                    expS = []
                    for p in range(NPAIR):
                        et = ep.tile([M, 2, CHUNK], BF, name=f"expS{p}",
                                     tag="expS")
                        for j in range(2):
                            base = 64 * j
                            spt = ps.tile([M, CHUNK], F, name=f"s{p}{j}",
                                          tag="S")
                            nc.tensor.matmul(
                                spt[:],
                                kT_r[p][base:base + DH, 0:M],
                                qT_r[p][base:base + DH, :],
                                start=True, stop=True)
                            nc.scalar.activation(et[:, j, :], spt[:],
                                                 AF.Exp, scale=SCALE)
                        expS.append(et)

                    # ---- Zs = per-head sums via selector columns ----
                    zs_ps = ps.tile([H, CHUNK], F, name="zs_ps", tag="fz")
                    for h in range(H):
                        nc.tensor.matmul(zs_ps[:], z_r[:, H * h:H * (h + 1)],
                                         expS[h // 2][:, h % 2, :],
                                         start=(h == 0), stop=(h == H - 1))
                    lnz = cp.tile([H, CHUNK], F, name="lnz", tag="lnz")
                    nc.scalar.activation(lnz[:], zs_ps[:], AF.Ln)
                    rs_r = cp.tile([H, CHUNK], BF, name="rs_r", tag="rs_r")
                    nc.scalar.activation(rs_r[:], lnz[:], AF.Exp, scale=-1.0)

                    # ---- O pairs + Rb, normalize with one DVE op/pair ----
                    o_sb = []
                    for p in range(NPAIR):
                        opp = ps.tile([128, CHUNK], F, name=f"op{p}", tag="O")
                        nc.tensor.matmul(opp[0:64, :], v_r[:, 2 * p, :],
                                         expS[p][:, 0, :], start=True,
                                         stop=True)
                        nc.tensor.matmul(opp[64:128, :],
                                         v_r[:, 2 * p + 1, :],
                                         expS[p][:, 1, :], start=True,
                                         stop=True)
                        rb_ps = ps.tile([128, CHUNK], F, name=f"rb{p}",
                                        tag="fz")
                        nc.tensor.matmul(rb_ps[:], e_r[:, 128 * p:128 * (p + 1)],
                                         rs_r[:], start=True, stop=True)
                        rb_sb = cp.tile([128, CHUNK], BF, name=f"rbs{p}",
                                        tag="rb_sb")
                        if p % 2 == 0:
                            nc.scalar.copy(rb_sb[:], rb_ps[:])
                        else:
                            nc.vector.tensor_copy(rb_sb[:], rb_ps[:])
                        ot = cp.tile([128, CHUNK], BF, name=f"osb{p}",
                                     tag=f"osb{p}")
                        nc.vector.tensor_tensor(out=ot[:], in0=opp[:],
                                                in1=rb_sb[:],
                                                op=mybir.AluOpType.mult)
                        o_sb.append(ot)

                    # ---- final projection + store ----
                    for i in range(4):
                        fp = ps.tile([128, QD], F, name=f"fin{i}", tag="O")
                        for p in range(NPAIR):
                            nc.tensor.matmul(fp[:], o_sb[p][:, 128 * i:128 * (i + 1)],
                                             wo_r[p][:], start=(p == 0),
                                             stop=(p == NPAIR - 1 and not with_bias))
                        if with_bias:
                            nc.tensor.matmul(fp[:], ones_r[:].bitcast(F),
                                             bo_r[:].bitcast(F),
                                             start=False, stop=True)
                        ft = cp.tile([128, QD], F, name=f"fout{i}",
                                     tag=f"fout{i}")
                        if i % 2 == 0:
                            nc.vector.tensor_copy(ft[:], fp[:])
                        else:
                            nc.scalar.copy(ft[:], fp[:])
                        nc.gpsimd.dma_start(
                            out_d[b, t0 + 128 * i: t0 + 128 * (i + 1), :],
                            ft[:])

    _legalize_sync_waits(nc, mybir)
    return nc


def _get_module(with_bias):
    key = ("mod", with_bias)
    if key not in _cache:
        import concourse.bass as bass
        _cache[key] = _build(bass.Bass(), with_bias)
    return _cache[key]


def kernel(x, context, Wq, Wk, Wv, Wo, bo):
    import os
    import sys
    if os.environ.get("JAX_PLATFORMS") == "cpu" and "jax" not in sys.modules:
        del os.environ["JAX_PLATFORMS"]
    from concourse.bass_utils import run_bass_kernel_spmd

    x = np.ascontiguousarray(x, dtype=np.float32)
    context = np.ascontiguousarray(context, dtype=np.float32)
    with_bias = bool(np.any(bo))
    aux = _aux_arrays(x, context,
                      np.asarray(Wq, dtype=np.float32),
                      np.asarray(Wk, dtype=np.float32),
                      np.asarray(Wv, dtype=np.float32),
                      np.asarray(Wo, dtype=np.float32),
                      np.asarray(bo, dtype=np.float32))
    nc = _get_module(with_bias)

    in_maps = []
    for core in range(N_CORES):
        sl = slice(B_LOC * core, B_LOC * (core + 1))
        m = {"xt": np.ascontiguousarray(aux["xt"][sl]),
             "ctp": np.ascontiguousarray(aux["ctp"][sl])}
        for k in ("aux_wq", "aux_wk", "aux_wv", "aux_wo", "aux_e", "aux_z",
                  "aux_bo"):
            m[k] = aux[k]
        in_maps.append(m)

    res = run_bass_kernel_spmd(nc, in_maps, core_ids=list(range(N_CORES)))
    return np.concatenate([r["out"] for r in res.results], axis=0)


# revision 39
# speedup vs baseline: 1.0035x; 1.0035x over previous
"""Cross-attention kernel for 8 Trainium2 NeuronCores.

Problem: out = softmax((x@Wq)(ctx@Wk)^T * dh^-0.5) @ (ctx@Wv) @ Wo + bo
  x [16, 4096, 320], ctx [16, 77, 768], H=8 heads x DH=40.

Sharding: data-parallel over batch (2 per core), SPMD one NEFF.

Layout ("^T domain": features on partitions, tokens on free dim), bf16
on-chip with fp32 PSUM accumulation:
  x^T, ctx^T   <- transposed + bf16-converted ON HOST, DMA'd directly
  q^T          <- Wq.T @ x^T, dense 3-group tiles {128,128,64}
  S_h          <- k^T_h.T @ q^T_h  [77 ctx-tok, 512 q-tok], pairs share
                  a 2-bank PSUM tile -> one exp per pair (ACT, bf16 out)
  Zs           <- accumulated selector-column matmuls [8, 512]
  Rs           <- exp(-ln(Zs))     (ACT; 1/Z without reciprocal)
  Rb           <- E_pair.T @ Rs    broadcast R rows to pair layout (PSUM)
  O_pair       <- v_h.T @ P_h (64-aligned pair rows), then ONE DVE
                  tensor_tensor per pair reading both PSUM operands:
                  o_sb = O_pair * Rb  (normalized, bf16)
  out          <- o_sb.T @ Wo_pad (+ bo) -> [tokens, 320] fp32 -> DMA

Matmul cost on PE is out-free-rows; bf16 keeps 1 cyc/row everywhere and
halves DMA traffic for x; final copies ride the idle Pool engine.
"""

import numpy as np

H, DH = 8, 40
SCALE = DH ** -0.5
B, N, M = 16, 4096, 77
QD, CD, ID = 320, 768, H * DH
N_CORES = 8
B_LOC = B // N_CORES
CHUNK = 512
NCHUNK = N // CHUNK
NPAIR = H // 2
MP = 80  # ctx tokens padded

_cache = {}


def _legalize_sync_waits(nc, mybir):
    """This walrus build allows 1 sync-wait command per instruction (2 for
    EventSemaphore). Spill extra waits onto same-engine NoOps placed just
    before; per-engine program order makes that equivalent."""
    n = 0
    f = nc.m.functions[0]
    for blk in f.blocks:
        out = []
        changed = False
        for inst in blk.instructions:
            si = inst.sync_info
            waits = list(si.on_wait) if si is not None and si.on_wait else []
            cap = 2 if isinstance(inst, mybir.InstEventSemaphore) else 1
            if len(waits) > cap:
                keep, spill = waits[-cap:], waits[:-cap]
                for w in spill:
                    n += 1
                    nop = mybir.InstNoOp(name=f"I-waitfix-{n}", ins=[], outs=[],
                                         engine=inst.engine)
                    nop.sync_info = mybir.SyncInfo(on_wait=[w], on_update=[])
                    out.append(nop)
                inst.sync_info = mybir.SyncInfo(
                    on_wait=keep,
                    on_update=list(si.on_update) if si.on_update else [])
                changed = True
            out.append(inst)
        if changed:
            blk.instructions = out
    return n


def _bf16(a):
    import ml_dtypes
    return np.asarray(a, dtype=np.float32).astype(ml_dtypes.bfloat16)


def _aux_arrays(x, context, Wq, Wk, Wv, Wo, bo):
    """Host-side prep: transpose x/ctx, convert weights, pair maps."""
    xt = _bf16(np.ascontiguousarray(np.transpose(x, (0, 2, 1))))  # [B,320,4096]
    ctp = np.zeros((B, CD, MP), dtype=np.float32)
    ctp[:, :, :M] = np.transpose(context, (0, 2, 1))

    def pad_pairs_cols(W):
        # W [c, 320] -> [c, 4, 128]: cols 0:40 = head 2p, 64:104 = head 2p+1
        c = W.shape[0]
        out = np.zeros((c, NPAIR, 128), dtype=np.float32)
        for p in range(NPAIR):
            out[:, p, 0:40] = W[:, 80 * p: 80 * p + 40]
            out[:, p, 64:104] = W[:, 80 * p + 40: 80 * p + 80]
        return out.reshape(c, NPAIR * 128)

    # E: [8, 4*128] broadcast map R_h -> pair rows
    e_mat = np.zeros((H, NPAIR * 128), dtype=np.float32)
    for p in range(NPAIR):
        e_mat[2 * p, 128 * p: 128 * p + 40] = 1.0
        e_mat[2 * p + 1, 128 * p + 64: 128 * p + 104] = 1.0
    # selector columns for Zs accumulation: [77, 8], col h = ones
    zcol = np.zeros((M, H, H), dtype=np.float32)
    for h in range(H):
        zcol[:, h, h] = 1.0
    zcol = zcol.reshape(M, H * H)
    # Wo rows in pair layout: [4, 128, 320]
    wo_pad = np.zeros((NPAIR, 128, QD), dtype=np.float32)
    for p in range(NPAIR):
        wo_pad[p, 0:40] = Wo[80 * p: 80 * p + 40]
        wo_pad[p, 64:104] = Wo[80 * p + 40: 80 * p + 80]
    return {
        "xt": xt, "ctp": _bf16(ctp),
        "aux_wq": _bf16(pad_pairs_cols(Wq)), "aux_wk": _bf16(pad_pairs_cols(Wk)),
        "aux_wv": _bf16(Wv),
        "aux_wo": _bf16(wo_pad.reshape(NPAIR * 128, QD)),
        "aux_e": _bf16(e_mat), "aux_z": _bf16(zcol),
        "aux_bo": bo.reshape(1, QD).astype(np.float32),
    }


def _build(nc, with_bias):
    import concourse.mybir as mybir
    from concourse.tile import TileContext

    F = mybir.dt.float32
    BF = mybir.dt.bfloat16
    AF = mybir.ActivationFunctionType

    xt_d = nc.dram_tensor("xt", [B_LOC, QD, N], BF, kind="ExternalInput")
    ct_d = nc.dram_tensor("ctp", [B_LOC, CD, MP], BF, kind="ExternalInput")
    wq_d = nc.dram_tensor("aux_wq", [QD, 512], BF, kind="ExternalInput")
    wk_d = nc.dram_tensor("aux_wk", [CD, 512], BF, kind="ExternalInput")
    wv_d = nc.dram_tensor("aux_wv", [CD, QD], BF, kind="ExternalInput")
    wo_d = nc.dram_tensor("aux_wo", [512, QD], BF, kind="ExternalInput")
    e_d = nc.dram_tensor("aux_e", [H, 512], BF, kind="ExternalInput")
    z_d = nc.dram_tensor("aux_z", [M, H * H], BF, kind="ExternalInput")
    bo_d = nc.dram_tensor("aux_bo", [1, QD], F, kind="ExternalInput")
    out_d = nc.dram_tensor("out", [B_LOC, N, QD], F, kind="ExternalOutput")

    GT = [(0, 128), (128, 128), (256, 64)]          # QD group/k tiles
    CKT6 = [(128 * i, 128) for i in range(6)]       # CD k-tiles

    with TileContext(nc) as tc:
        with tc.tile_pool(name="wpool", bufs=1) as wp, \
             tc.tile_pool(name="bpool", bufs=2) as bp, \
             tc.tile_pool(name="cpool", bufs=2) as cp, \
             tc.tile_pool(name="epool", bufs=10) as ep, \
             tc.tile_pool(name="ps", bufs=2, space="PSUM") as ps:

            # ---- per-core constants ----
            wq_r = []
            for kt, (o, w) in enumerate(GT):
                t = wp.tile([w, 512], BF, name=f"wq{kt}", tag=f"wq{kt}")
                nc.sync.dma_start(t[:], wq_d[o:o + w, :])
                wq_r.append(t)
            ctx_tiles = []
            for b in range(B_LOC):
                t = bp.tile([128, 6, MP], BF, name=f"ctx{b}", tag="ctx")
                nc.sync.dma_start(
                    t[:], ct_d[b].rearrange("(k p) m -> p k m", p=128))
                ctx_tiles.append(t)
            wk_r = []
            wv_r = []
            for kt, (o, w) in enumerate(CKT6):
                t = wp.tile([w, 512], BF, name=f"wk{kt}", tag=f"wk{kt}")
                nc.gpsimd.dma_start(t[:], wk_d[o:o + w, :])
                wk_r.append(t)
                t2 = wp.tile([w, QD], BF, name=f"wv{kt}", tag=f"wv{kt}")
                nc.gpsimd.dma_start(t2[:], wv_d[o:o + w, :])
                wv_r.append(t2)
            wo_r = []
            for p in range(NPAIR):
                t = wp.tile([128, QD], BF, name=f"wo{p}", tag=f"wo{p}")
                nc.gpsimd.dma_start(t[:], wo_d[128 * p:128 * p + 128, :])
                wo_r.append(t)
            e_r = wp.tile([H, 512], BF, name="e_r", tag="e_r")
            nc.gpsimd.dma_start(e_r[:], e_d[:])
            z_r = wp.tile([M, H * H], BF, name="z_r", tag="z_r")
            nc.gpsimd.dma_start(z_r[:], z_d[:])
            if with_bias:
                bo_r = wp.tile([1, QD], F, name="bo_r", tag="bo_r")
                nc.gpsimd.dma_start(bo_r[:], bo_d[:])
                ones_r = wp.tile([1, 128], F, name="ones_r", tag="ones_r")
                nc.vector.memset(ones_r[:], 1.0)

            def batch_prep(b):
                # ---- per-batch: k^T pairs, v ----
                ctx_f = ctx_tiles[b]
                kT_r = []
                for p in range(NPAIR):
                    pt = ps.tile([128, MP], F, name=f"kps{p}", tag="q")
                    for kt in range(6):
                        nc.tensor.matmul(pt[:], wk_r[kt][:, 128 * p:128 * (p + 1)],
                                         ctx_f[:, kt, :], start=(kt == 0),
                                         stop=(kt == 5))
                    st = bp.tile([128, MP], BF, name=f"kT{p}", tag=f"kT{p}")
                    nc.vector.tensor_copy(st[:], pt[:])
                    kT_r.append(st)

                v_ps = ps.tile([MP, ID], F, name="v_ps", tag="q")
                for kt in range(6):
                    nc.tensor.matmul(v_ps[:], ctx_f[:, kt, 0:MP], wv_r[kt][:],
                                     start=(kt == 0), stop=(kt == 5))
                v_r = bp.tile([M, H, 64], BF, name="v_r", tag="v_r")
                nc.vector.memset(v_r[:, :, 40:64], 0.0)
                nc.vector.tensor_copy(
                    v_r[:, :, 0:40],
                    v_ps[0:M, :].rearrange("p (h d) -> p h d", h=H))
                return kT_r, v_r

            prep = {0: batch_prep(0)}
            for b in range(B_LOC):
                kT_r, v_r = prep[b]
                for c in range(NCHUNK):
                    t0 = CHUNK * c
                    if c == 0 and b + 1 < B_LOC:
                        prep[b + 1] = batch_prep(b + 1)
                    # ---- x^T tiles straight from HBM ----
                    xT_r = []
                    for g, (go, gw) in enumerate(GT):
                        st = cp.tile([gw, CHUNK], BF, name=f"xT{g}",
                                     tag=f"xT{g}")
                        nc.sync.dma_start(st[:], xt_d[b, go:go + gw,
                                                      t0:t0 + CHUNK])
                        xT_r.append(st)

                    # ---- q^T head pairs ----
                    qT_r = []
                    for p in range(NPAIR):
                        pt = ps.tile([128, CHUNK], F, name=f"qps{p}", tag="q")
                        for kt in range(3):
                            nc.tensor.matmul(pt[:], wq_r[kt][:, 128 * p:128 * (p + 1)],
                                             xT_r[kt][:], start=(kt == 0),
                                             stop=(kt == 2))
                        st = cp.tile([128, CHUNK], BF, name=f"qT{p}",
                                     tag=f"qT{p}")
                        nc.vector.tensor_copy(st[:], pt[:])
                        qT_r.append(st)

                    # ---- scores + exp per head ----
                    expS = []
                    for p in range(NPAIR):
                        et = ep.tile([M, 2, CHUNK], BF, name=f"expS{p}",
                                     tag="expS")
                        for j in range(2):
                            base = 64 * j
                            spt = ps.tile([M, CHUNK], F, name=f"s{p}{j}",
                                          tag="S")
                            nc.tensor.matmul(
                                spt[:],
                                kT_r[p][base:base + DH, 0:M],
                                qT_r[p][base:base + DH, :],
                                start=True, stop=True)
                            nc.scalar.activation(et[:, j, :], spt[:],
                                                 AF.Exp, scale=SCALE)
                        expS.append(et)

                    # ---- Zs = per-head sums via selector columns ----
                    zs_ps = ps.tile([H, CHUNK], F, name="zs_ps", tag="fz")
                    for h in range(H):
                        nc.tensor.matmul(zs_ps[:], z_r[:, H * h:H * (h + 1)],
                                         expS[h // 2][:, h % 2, :],
                                         start=(h == 0), stop=(h == H - 1))
                    lnz = cp.tile([H, CHUNK], F, name="lnz", tag="lnz")
                    nc.scalar.activation(lnz[:], zs_ps[:], AF.Ln)
                    rs_r = cp.tile([H, CHUNK], BF, name="rs_r", tag="rs_r")
                    nc.scalar.activation(rs_r[:], lnz[:], AF.Exp, scale=-1.0)

                    # ---- O pairs + Rb, normalize with one DVE op/pair ----
                    o_sb = []
                    for p in range(NPAIR):
                        opp = ps.tile([128, CHUNK], F, name=f"op{p}", tag="O")
                        nc.tensor.matmul(opp[0:64, :], v_r[:, 2 * p, :],
                                         expS[p][:, 0, :], start=True,
                                         stop=True)
                        nc.tensor.matmul(opp[64:128, :],
                                         v_r[:, 2 * p + 1, :],
                                         expS[p][:, 1, :], start=True,
                                         stop=True)
                        rb_ps = ps.tile([128, CHUNK], F, name=f"rb{p}",
                                        tag="fz")
                        nc.tensor.matmul(rb_ps[:], e_r[:, 128 * p:128 * (p + 1)],
                                         rs_r[:], start=True, stop=True)
                        rb_sb = cp.tile([128, CHUNK], BF, name=f"rbs{p}",
                                        tag="rb_sb")
                        if p % 2 == 0:
                            nc.scalar.copy(rb_sb[:], rb_ps[:])
                        else:
                            nc.vector.tensor_copy(rb_sb[:], rb_ps[:])
                        ot = cp.tile([128, CHUNK], BF, name=f"osb{p}",
                                     tag=f"osb{p}")
                        nc.vector.tensor_tensor(out=ot[:], in0=opp[:],
                                                in1=rb_sb[:],
                                                op=mybir.AluOpType.mult)
                        o_sb.append(ot)

                    # ---- final projection + store ----
                    for i in range(4):
                        fp = ps.tile([128, QD], F, name=f"fin{i}", tag="O")
                        for p in range(NPAIR):
                            nc.tensor.matmul(fp[:], o_sb[p][:, 128 * i:128 * (i + 1)],
                                             wo_r[p][:], start=(p == 0),
                                             stop=(p == NPAIR - 1 and not with_bias))
                        if with_bias:
                            nc.tensor.matmul(fp[:], ones_r[:].bitcast(F),
                                             bo_r[:].bitcast(F),
                                             start=False, stop=True)
                        ft = cp.tile([128, QD], F, name=f"fout{i}",
                                     tag=f"fout{i}")
                        if i % 2 == 0:
                            nc.vector.tensor_copy(ft[:], fp[:])
                        else:
                            nc.scalar.copy(ft[:], fp[:])
                        nc.gpsimd.dma_start(
                            out_d[b, t0 + 128 * i: t0 + 128 * (i + 1), :],
                            ft[:])

    _legalize_sync_waits(nc, mybir)
    return nc


def _get_module(with_bias):
    key = ("mod", with_bias)
    if key not in _cache:
        import concourse.bass as bass
        _cache[key] = _build(bass.Bass(), with_bias)
    return _cache[key]


def kernel(x, context, Wq, Wk, Wv, Wo, bo):
    import os
    import sys
    if os.environ.get("JAX_PLATFORMS") == "cpu" and "jax" not in sys.modules:
        del os.environ["JAX_PLATFORMS"]
    from concourse.bass_utils import run_bass_kernel_spmd

    x = np.ascontiguousarray(x, dtype=np.float32)
    context = np.ascontiguousarray(context, dtype=np.float32)
    with_bias = bool(np.any(bo))
    aux = _aux_arrays(x, context,
                      np.asarray(Wq, dtype=np.float32),
                      np.asarray(Wk, dtype=np.float32),
                      np.asarray(Wv, dtype=np.float32),
                      np.asarray(Wo, dtype=np.float32),
                      np.asarray(bo, dtype=np.float32))
    nc = _get_module(with_bias)

    in_maps = []
    for core in range(N_CORES):
        sl = slice(B_LOC * core, B_LOC * (core + 1))
        m = {"xt": np.ascontiguousarray(aux["xt"][sl]),
             "ctp": np.ascontiguousarray(aux["ctp"][sl])}
        for k in ("aux_wq", "aux_wk", "aux_wv", "aux_wo", "aux_e", "aux_z",
                  "aux_bo"):
            m[k] = aux[k]
        in_maps.append(m)

    res = run_bass_kernel_spmd(nc, in_maps, core_ids=list(range(N_CORES)))
    return np.concatenate([r["out"] for r in res.results], axis=0)
